# revision 25
# baseline (speedup 1.0000x reference)
"""Trainium2 Bass kernel: fused concat-linear attention map + softmax.

reference:  scores[b,h,n] = key[b,n,:]@Wk[h,:] + query[b,0,:]@Wq[h,:] + bias[h]
            attn = softmax over n              (B=16, N=20000, D=256, H=8)

Sharding: batch dim B=16 split across 8 cores (2 batches/core), weights
replicated.

Per-core design (v2): avoid the PE-transpose-per-128x128-block pipeline
(which saturated the tensor engine at ~300us) by
  1. p-major loads: ld[p, s, d] = key[n0 + p*S + s, d] -> each of the 128
     partition lines is S*1KB contiguous in HBM (16KB DMA descriptors,
     near-line-rate ~358 GB/s).
  2. One DVE stream-transpose per load ([128, S*256], 32x32 blocks):
     ldt[32a+v, (s,b,u)] = key[n0+(32a+u)*S+s, 32b+v] -- d%32 moves onto
     partitions.
  3. 8 accumulating f32r matmuls with block-diagonal delta-weights
     lhsT_b[(a,v), 8a'+h] = (a==a') * Wk[h, 32b+v] compute scores for all
     128*S rows into ONE [32, S*32] PSUM bank (output partition = 8a'+h).
  4. One ScalarE exp per load (bias = q-term+b), writing prob with a
     strided AP so each partition holds a contiguous n-run; accum_out
     gives per-load softmax partial sums.
  5. Softmax normalization via tiny reduction matmuls + ACT copy-scale;
     contiguous output DMA per a'-group.
"""

import sys

import numpy as np

for _p in ("/opt/trn_rl_repo",):
    if _p not in sys.path:
        sys.path.append(_p)

from contextlib import ExitStack

import concourse.bass as bass
import concourse.bacc as bacc
import concourse.tile as tile
from concourse import mybir
from concourse.masks import make_identity

B, N, D, H = 16, 20000, 256, 8
NCORES = 8
BPC = B // NCORES  # batches per core
P = 128
F32 = mybir.dt.float32
F32R = mybir.dt.float32r
BF16 = mybir.dt.bfloat16
I32 = mybir.dt.int32

# load plan: per batch, 4 superloads of S=32 rows/partition-line + one of
# S=28 (128*S rows each), + a 32-row tail. Each superload is 4 DMAs (one per
# 32-partition a-block) of 32 x S KB with S*1KB-contiguous descriptor lines.
LOADS = [32] * 4 + [28]  # S per load; rows = 128*S
MAIN_ROWS = sum(128 * s for s in LOADS)  # 19968
TAIL_ROWS = N - MAIN_ROWS  # 32


def _r(ap):
    return ap.bitcast(F32R)


def build_kernel(n=N, bpc=BPC):
    nc = bacc.Bacc("TRN2", target_bir_lowering=False, debug=False)
    q_in = nc.declare_dram_parameter("q", [bpc, D], F32, isOutput=False)
    k_in = nc.declare_dram_parameter("k", [bpc, n, D], F32, isOutput=False)
    w_in = nc.declare_dram_parameter("w", [H, 2 * D], F32, isOutput=False)
    b_in = nc.declare_dram_parameter("b", [H], F32, isOutput=False)
    out = nc.declare_dram_parameter("out", [bpc, H, n], F32, isOutput=True)

    ncols_main = sum(32 * s for s in LOADS)  # per-partition prob cols (4992)

    with ExitStack() as ctx:
        tc = ctx.enter_context(tile.TileContext(nc))
        consts = ctx.enter_context(tc.tile_pool(name="consts", bufs=1))
        loads = ctx.enter_context(tc.tile_pool(name="loads", bufs=4))
        ldts = ctx.enter_context(tc.tile_pool(name="ldts", bufs=3))
        probp = ctx.enter_context(tc.tile_pool(name="prob", bufs=2))
        small = ctx.enter_context(tc.tile_pool(name="small", bufs=2))
        psum_sc = ctx.enter_context(tc.tile_pool(name="psum_sc", bufs=3, space="PSUM"))
        psum_mi = ctx.enter_context(tc.tile_pool(name="psum_mi", bufs=2, space="PSUM"))

        identity = consts.tile([P, P], F32)
        make_identity(nc, identity)

        # --- constants ------------------------------------------------------
        w_sb = consts.tile([H, 2 * D], F32)
        nc.sync.dma_start(out=w_sb[:, :], in_=w_in[:, :])
        b_sb = consts.tile([H, 1], F32)
        nc.sync.dma_start(out=b_sb[:, :], in_=b_in[:])
        q_sb = consts.tile([1, bpc, D], F32)
        nc.sync.dma_start(out=q_sb[:, :, :], in_=q_in[:, :])

        # wqT[:, c, :]: Wq halves transposed to [d, h]
        wqT = consts.tile([P, 2, H], F32)
        for c in range(2):
            pt = psum_mi.tile([P, H], F32, tag="mi")
            nc.tensor.transpose(pt[:, :], w_sb[:, c * P:(c + 1) * P], identity[:H, :H])
            nc.vector.tensor_copy(out=wqT[:, c, :], in_=pt[:, :])

        # delta[(a,vp), j=(b32,e), 8a+h] = Wk[h, 64*b32 + 2*vp + e]
        # (the int32-pair stream transpose leaves d%2 in the free dim)
        delta32 = consts.tile([P, 8, 32], F32)
        nc.vector.memset(delta32[:, :, :], 0.0)
        wkT_small = consts.tile([32, 8, H], BF16)
        for j in range(8):
            b32, e = j // 2, j % 2
            pt = psum_mi.tile([32, H], F32, tag="mi")
            wk_slice = w_sb[:, D + 64 * b32:D + 64 * (b32 + 1)].rearrange(
                "h (vp e) -> h e vp", e=2
            )[:, e, :]
            nc.tensor.transpose(pt[:, :], wk_slice, identity[:H, :H])
            nc.vector.tensor_copy(out=wkT_small[:, j, :], in_=pt[:, :])
            for a in range(4):
                nc.vector.tensor_copy(
                    out=delta32[32 * a:32 * (a + 1), j, 8 * a:8 * (a + 1)],
                    in_=pt[:, :],
                )
        delta = consts.tile([P, 8, 32], BF16)
        nc.vector.tensor_copy(out=delta[:, :, :], in_=delta32[:, :, :])

        # E[h', 8a+h] = (h'==h): [I8 I8 I8 I8]  (f32: feeds tiny f32 matmuls)
        E = consts.tile([H, 32], F32)
        for a in range(4):
            nc.vector.tensor_copy(out=E[:, 8 * a:8 * (a + 1)], in_=identity[:H, :H])
        # F[(8a+h), h'] = (h==h'): 4 stacked I8 blocks, via transpose of E
        F = consts.tile([32, H], F32)
        ptF = psum_mi.tile([32, H], F32, tag="mi")
        nc.tensor.transpose(ptF[:, :], E[:, :], identity[:H, :H])
        nc.vector.tensor_copy(out=F[:, :], in_=ptF[:, :])

        # qT: query transposed to [d, i, c]
        qT = consts.tile([P, bpc, 2], F32)
        for i in range(bpc):
            for c in range(2):
                pt = psum_mi.tile([P, 1], F32, tag="mi")
                nc.tensor.transpose(
                    pt[:, :], q_sb[0:1, i, c * P:(c + 1) * P], identity[:1, :1]
                )
                nc.vector.tensor_copy(out=qT[:, i, c:c + 1], in_=pt[:, :])

        # qb8[h, i] = Wq @ q_i + bias
        qb8 = consts.tile([H, bpc], F32)
        for i in range(bpc):
            qp = psum_mi.tile([H, 1], F32, tag="mi")
            nc.tensor.matmul(
                qp[:, :], wqT[:, 0, :], qT[:, i, 0:1], start=True, stop=False
            )
            nc.tensor.matmul(
                qp[:, :], wqT[:, 1, :], qT[:, i, 1:2], start=False, stop=True
            )
            nc.vector.tensor_add(qb8[:, i:i + 1], qp[:, :], b_sb[:, :])
        # qb32[8a+h, i] = qb8[h, i]
        qb32 = consts.tile([32, bpc], F32)
        qp32 = psum_mi.tile([32, bpc], F32, tag="mi")
        nc.tensor.matmul(qp32[:, :], E[:, :], qb8[:, :], start=True, stop=True)
        nc.vector.tensor_copy(out=qb32[:, :], in_=qp32[:, :])

        # --- main loop ------------------------------------------------------
        for i in range(bpc):
            prob = probp.tile([32, ncols_main], F32, tag="prob")
            prob_t = probp.tile([H, TAIL_ROWS], F32, tag="probt")
            sums = small.tile([32, 2 * len(LOADS)], F32, tag="sums")
            sums_t = small.tile([H, 1], F32, tag="sumst")

            # row mapping: n = a*ncols_main + c0 + u*S + s, so each prob
            # partition (8a+h) ends up with one contiguous n-range.
            kv = k_in[i, 0:4 * ncols_main, :].rearrange(
                "(a c) d -> a c d", a=4
            )
            c0 = 0
            for L, S in enumerate(LOADS):
                ld = loads.tile([P, S, D], BF16, tag="load")
                for a in range(4):
                    nc.gpsimd.dma_start(
                        out=ld[32 * a:32 * (a + 1), :, :],
                        in_=kv[a, c0:c0 + 32 * S, :].rearrange(
                            "(u s) d -> u s d", s=S
                        ),
                    )
                ldt = ldts.tile([P, S, 4, 32, 2], BF16, tag="ldt")
                nc.vector.transpose(
                    out=ldt[:, :, :, :, :].rearrange(
                        "p s b u e -> p (s b u e)"
                    ).bitcast(I32),
                    in_=ld[:, :, :].rearrange("p s d -> p (s d)").bitcast(I32),
                )
                # two PSUM slices (s-halves) per superload
                hs = S // 2
                for k in range(2):
                    s0 = k * hs
                    scp = psum_sc.tile([32, hs, 32], F32, tag="sc")
                    for j in range(8):
                        b32, e = j // 2, j % 2
                        nc.tensor.matmul(
                            scp[:, :, :],
                            delta[:, j, :],
                            ldt[:, s0:s0 + hs, b32, :, e],
                            start=(j == 0),
                            stop=(j == 7),
                        )
                    # prob cols: c0 + u*S + s (natural n order per partition)
                    nc.scalar.activation(
                        out=prob[:, c0:c0 + 32 * S].rearrange(
                            "p (u s) -> p s u", s=S
                        )[:, s0:s0 + hs, :],
                        in_=scp[:, :, :],
                        func=mybir.ActivationFunctionType.Exp,
                        bias=qb32[:, i:i + 1],
                        scale=1.0,
                        accum_out=sums[:, 2 * L + k:2 * L + k + 1],
                    )
                c0 += 32 * S

            # 32-row tail: natural layout, K=32 matmuls on partitions 0-7
            ld_tb = loads.tile([TAIL_ROWS, D], BF16, tag="loadtb")
            nc.gpsimd.dma_start(out=ld_tb[:, :], in_=k_in[i, 4 * ncols_main:n, :])
            ldt_tb = ldts.tile([TAIL_ROWS, 4, 32, 2], BF16, tag="ldttb")
            nc.vector.transpose(
                out=ldt_tb[:, :, :, :].rearrange("p b u e -> p (b u e)").bitcast(I32),
                in_=ld_tb[:, :].bitcast(I32),
            )
            sct = psum_sc.tile([H, TAIL_ROWS], F32, tag="sct")
            for j in range(8):
                b32, e = j // 2, j % 2
                nc.tensor.matmul(
                    sct[:, :],
                    wkT_small[:, j, :],
                    ldt_tb[:, b32, :, e],
                    start=(j == 0),
                    stop=(j == 7),
                )
            nc.scalar.activation(
                out=prob_t[:, :],
                in_=sct[:, :],
                func=mybir.ActivationFunctionType.Exp,
                bias=qb8[:, i:i + 1],
                scale=1.0,
                accum_out=sums_t[:, :],
            )

            # totals: tot8[h] = sum_a sums[(8a+h), :] + sums_t[h]  (all-f32 tinies)
            sums_r = small.tile([32, 1], F32, tag="sumsr")
            nc.vector.reduce_sum(
                out=sums_r[:, :], in_=sums[:, :], axis=mybir.AxisListType.X
            )
            tot8 = psum_mi.tile([H, 1], F32, tag="mi")
            nc.tensor.matmul(
                tot8[:, :], F[:, :], sums_r[:, :], start=True, stop=False
            )
            nc.tensor.matmul(
                tot8[:, :], identity[:H, :H], sums_t[:, :], start=False, stop=True
            )
            rec8 = small.tile([H, 1], F32, tag="rec8")
            nc.vector.reciprocal(out=rec8[:, :], in_=tot8[:, :])
            rec32p = psum_mi.tile([32, 1], F32, tag="mi")
            nc.tensor.matmul(
                rec32p[:, :], E[:, :], rec8[:, :], start=True, stop=True
            )
            rec32 = small.tile([32, 1], F32, tag="rec32")
            nc.vector.tensor_copy(out=rec32[:, :], in_=rec32p[:, :])

            # normalize + output: out[i, h, a*ncols_main + m] = prob[8a+h, m]/Z.
            # One DMA per a-block (DMA APs only honor a single leading
            # partition dim), chunked so DVE scale and out-DMA pipeline.
            half = 2560
            nc.vector.tensor_scalar_mul(
                prob[:, 0:half], prob[:, 0:half], rec32[:, :]
            )
            for a in range(4):
                eng = nc.sync if a % 2 == 0 else nc.scalar
                eng.dma_start(
                    out=out[i, :, a * ncols_main:a * ncols_main + half],
                    in_=prob[8 * a:8 * (a + 1), 0:half],
                )
            nc.vector.tensor_scalar_mul(
                prob[:, half:], prob[:, half:], rec32[:, :]
            )
            nc.vector.tensor_scalar_mul(prob_t[:, :], prob_t[:, :], rec8[:, :])
            for a in range(4):
                eng = nc.scalar if a % 2 == 0 else nc.sync
                eng.dma_start(
                    out=out[i, :, a * ncols_main + half:(a + 1) * ncols_main],
                    in_=prob[8 * a:8 * (a + 1), half:],
                )
            nc.sync.dma_start(out=out[i, :, 4 * ncols_main:n], in_=prob_t[:, :])

    nc.compile()
    return nc


_NC_CACHE = {}


def _get_nc():
    if "nc" not in _NC_CACHE:
        _NC_CACHE["nc"] = build_kernel()
    return _NC_CACHE["nc"]


def kernel(query, key, W, b):
    from concourse.bass_utils import run_bass_kernel_spmd

    query = np.ascontiguousarray(np.asarray(query, np.float32).reshape(B, D))
    key = np.ascontiguousarray(np.asarray(key, np.float32))
    W = np.ascontiguousarray(np.asarray(W, np.float32))
    b = np.ascontiguousarray(np.asarray(b, np.float32))

    nc = _get_nc()
    in_maps = []
    for c in range(NCORES):
        s = slice(BPC * c, BPC * (c + 1))
        in_maps.append(
            {
                "q": query[s],
                "k": key[s],
                "w": W,
                "b": b,
            }
        )
    res = run_bass_kernel_spmd(nc, in_maps, list(range(NCORES))).results
    return np.concatenate([res[c]["out"] for c in range(NCORES)], axis=0)
